# revision 37
# baseline (speedup 1.0000x reference)
"""DualAttention Trainium2 kernel (nn_DualAttention_44341242364496).

Reference math (per batch element, X = points[b], shape (N=4096, C=256)):
  q = X Wq^T + bq ; k = X Wk^T + bk          (N, 32)
  P = softmax(q k^T, axis=-1)                (N, N)
  v = X Wv^T + bv                            (N, 256)
  out_p = gamma * P v + X
  E = X X^T contracted over rows -> (C, C):  E = X^T X
  A = softmax(max_d(E) - E, axis=-1) == stable softmax(-E)
  out_c = gamma * A X^T' (einsum bcd,bnd->bnc) + X
  out = out_p + out_c = gamma*(Pv) + gamma*(X A^T) + 2X

Distribution: 8 cores; core c handles batch b=c//2, query-row half h=c%2.
Each core receives its batch's X in two layouts (scaled by 2, with the
core's own rows ordered first), computes k/v/E for the full batch element
(duplicated with its pair core), and produces its 2048 output rows.

Softmax tricks used on-chip:
 - point branch: exp() without max subtraction (scores are O(+-40), safe in
   fp32), denominator obtained by appending a ones-column to v so the PV
   matmul also yields row sums; division folded into the output scaling.
 - channel branch: softmax(max-E) == exp(-(E - min_row)) / sum.
All matmuls run in float32r (TF32-like PE fast path, 1 cycle/row).
"""

import sys

sys.path.insert(0, "/opt/trn_rl_repo")

import numpy as np
import ml_dtypes

import concourse.bass as bass  # noqa: F401  (bass types used via bacc/tile)
import concourse.mybir as mybir
import concourse.tile as tile
from concourse import bacc
from concourse.bass_utils import run_bass_kernel_spmd
from concourse.masks import make_identity

B, N, C = 4, 4096, 256
C8 = C // 8  # 32
NCORES = 8
HALF = N // 2  # 2048 query rows per core
NBLK = HALF // 128  # 16 output row blocks per core
KCH = N // 128  # 32 key chunks
P = 128

F32 = mybir.dt.float32
F32R = mybir.dt.float32r
BF16 = mybir.dt.bfloat16
F16 = mybir.dt.float16
AX = mybir.AxisListType
ALU = mybir.AluOpType
ACTF = mybir.ActivationFunctionType

_CACHE: dict = {}


def _build_nc():
    nc = bacc.Bacc("TRN2", target_bir_lowering=False)

    x2T_d = nc.dram_tensor("x2T", [P, 2, N], F16, kind="ExternalInput")
    x2_d = nc.dram_tensor("x2", [P, KCH, C], F16, kind="ExternalInput")
    wqT_d = nc.dram_tensor("wqT", [P, 2, C8], F16, kind="ExternalInput")
    wkT_d = nc.dram_tensor("wkT", [P, 2, C8], F16, kind="ExternalInput")
    wvT_d = nc.dram_tensor("wvT", [P, 2, C + 2], F16, kind="ExternalInput")
    bq_d = nc.dram_tensor("bqc2", [2 * C8, 1], F32, kind="ExternalInput")
    bk_d = nc.dram_tensor("bkc2", [2 * C8, 1], F32, kind="ExternalInput")
    bv_d = nc.dram_tensor("bvr", [1, C + 2], F32, kind="ExternalInput")
    gam_d = nc.dram_tensor("gam", [1, 1], F32, kind="ExternalInput")
    out_d = nc.dram_tensor("out_rows", [NBLK, P, C], F32, kind="ExternalOutput")

    with tile.TileContext(nc) as tc:
        with (
            tc.tile_pool(name="singles", bufs=1) as singles,
            tc.tile_pool(name="persist", bufs=1) as persist,
            tc.tile_pool(name="pTp", bufs=3) as pTp,
            tc.tile_pool(name="sbout", bufs=3) as sbout,
            tc.tile_pool(name="small", bufs=10) as small,
            tc.tile_pool(name="psS", bufs=2, space="PSUM") as psS,
            tc.tile_pool(name="psO", bufs=4, space="PSUM") as psO,
        ):
            # ---------------- Phase A: loads & constants ----------------
            # x2 is split into chunk-group DMAs so energy matmuls can start
            # as soon as the first chunks land; weights go on the ACT HWDGE
            # queue so they don't queue behind the big SP transfers.
            x2 = persist.tile([P, KCH, C], F16, tag="x2")
            x2T = persist.tile([P, 2, N], F16, tag="x2T")
            # interleave the issue order so the first chunks of BOTH layouts
            # land early (energy needs x2 chunks, V/kq need x2T columns)
            for g in range(8):
                nc.sync.dma_start(
                    x2[:, g * 2 : (g + 1) * 2, :], x2_d.ap()[:, g * 2 : (g + 1) * 2, :]
                )
                for cc in range(2):
                    nc.sync.dma_start(
                        x2T[:, cc, g * 512 : (g + 1) * 512],
                        x2T_d.ap()[:, cc, g * 512 : (g + 1) * 512],
                    )
            for g in range(8, 16):
                nc.sync.dma_start(
                    x2[:, g * 2 : (g + 1) * 2, :], x2_d.ap()[:, g * 2 : (g + 1) * 2, :]
                )
            wqT = singles.tile([P, 2, C8], F16, tag="wqT")
            nc.scalar.dma_start(wqT[:], wqT_d.ap())
            wkT = singles.tile([P, 2, C8], F16, tag="wkT")
            nc.scalar.dma_start(wkT[:], wkT_d.ap())
            wvT = singles.tile([P, 2, C + 2], F16, tag="wvT")
            nc.scalar.dma_start(wvT[:], wvT_d.ap())
            bqc2 = singles.tile([2 * C8, 1], F32, tag="bqc2")
            nc.scalar.dma_start(bqc2[:], bq_d.ap())
            bkc2 = singles.tile([2 * C8, 1], F32, tag="bkc2")
            nc.scalar.dma_start(bkc2[:], bk_d.ap())
            bvb = singles.tile([P, C + 2], F32, tag="bvb")
            nc.gpsimd.dma_start(
                bvb[:],
                bass.AP(tensor=bv_d, offset=0, ap=[[0, P], [1, C + 2]]),
            )
            ident = singles.tile([P, P], F32, tag="ident")
            make_identity(nc, ident[:])
            gb = singles.tile([P, 1], F32, tag="gb")
            nc.scalar.dma_start(gb[:], gam_d.ap().to_broadcast([P, 1]))
            gh = singles.tile([P, 1], F32, tag="gh")
            nc.vector.tensor_scalar_mul(gh[:], gb[:], 0.5)

            # ------- Phase B: channel attention (E = X^T X, softmax) -------
            # E' = x2^T x2 = 4E ; A = exp(-(E - min)) / sum = exp(-.25 E' + .25 min')
            attn_n = singles.tile([P, 2, C], F32, tag="attn_n")
            attnTg = persist.tile([P, 2, C], F16, tag="attnTg")
            e_ps = [
                psO.tile([P, 512], F32, tag="o", name=f"e_{cb}")[:, :C]
                for cb in range(2)
            ]
            kT2 = persist.tile([2 * C8, N // 2], F16, tag="kT2")
            qT2 = persist.tile([2 * C8, HALF], F16, tag="qT2")
            vaug = persist.tile([P, KCH, C + 2], BF16, tag="vaug")

            def emit_energy(nk):
                for cb in range(2):
                    nc.tensor.matmul(
                        e_ps[cb],
                        x2[:, nk, cb * P : (cb + 1) * P],
                        x2[:, nk, :],
                        start=(nk == 0),
                        stop=(nk == KCH - 1),
                    )

            def emit_v(nk):
                # V psums ride the two spare psO slots during the phases
                vps = psO.tile([P, 512], F32, tag="o", name=f"v_{nk}")[:, : C + 2]
                for cc in range(2):
                    nc.tensor.matmul(
                        vps,
                        x2T[:, cc, nk * P : (nk + 1) * P],
                        wvT[:, cc, :],
                        start=(cc == 0),
                        stop=(cc == 1),
                    )
                nc.vector.tensor_add(vaug[:, nk, :], vps, bvb[:])

            def emit_k(quarter):
                # both groups land in one psum bank (col groups 0/32) so a
                # single [64,512] copy moves them into kT2
                kps = psS.tile([P, 1024], F32, tag="s", name=f"k_{quarter}")
                for grp in range(2):
                    for cc in range(2):
                        rhs = x2T[:, cc, :].rearrange(
                            "a (r two p) -> a two r p", two=2, p=P
                        )[:, grp, 4 * quarter : 4 * (quarter + 1), :]
                        nc.tensor.matmul(
                            kps[grp * C8 : (grp + 1) * C8, :512],
                            wkT[:, cc, :],
                            rhs,
                            start=(cc == 0),
                            stop=(cc == 1),
                        )
                nc.scalar.activation(
                    kT2[:, quarter * 512 : (quarter + 1) * 512],
                    kps[: 2 * C8, :512],
                    ACTF.Identity,
                    bias=bkc2[:],
                )

            def emit_q(seg):
                qps = psS.tile([P, 1024], F32, tag="s", name=f"q_{seg}")
                for grp in range(2):
                    for cc in range(2):
                        nc.tensor.matmul(
                            qps[grp * C8 : (grp + 1) * C8, :512],
                            wqT[:, cc, :],
                            x2T[:, cc, seg * 512 : (seg + 1) * 512],
                            start=(cc == 0),
                            stop=(cc == 1),
                        )
                nc.scalar.activation(
                    qT2[:, seg * 512 : (seg + 1) * 512],
                    qps[: 2 * C8, :512],
                    ACTF.Identity,
                    bias=bqc2[:],
                )

            # interleave: keeps PE dense while DVE (v-bias) and ACT
            # (kq copies) work in parallel rather than phase-by-phase
            kq_steps = [("k", qq) for qq in range(4)] + [
                ("q", sg) for sg in range(4)
            ]
            for nk in range(KCH):
                emit_energy(nk)
                emit_v(nk)
                if nk % 4 == 0:
                    kind, a = kq_steps[nk // 4]
                    if kind == "k":
                        emit_k(a)
                    else:
                        emit_q(a)
            for cb in range(2):
                emin = small.tile([P, 1], F32, tag="sm", name=f"emin{cb}")
                nc.vector.tensor_reduce(
                    emin[:], e_ps[cb], axis=AX.X, op=ALU.min
                )
                emq = small.tile([P, 1], F32, tag="sm", name=f"emq{cb}")
                nc.vector.tensor_scalar_mul(emq[:], emin[:], 0.25)
                us = small.tile([P, 1], F32, tag="sm", name=f"us{cb}")
                nc.scalar.activation(
                    attn_n[:, cb, :],
                    e_ps[cb],
                    ACTF.Exp,
                    bias=emq[:],
                    scale=-0.25,
                    accum_out=us[:],
                )
                rc = small.tile([P, 1], F32, tag="sm", name=f"rc{cb}")
                nc.vector.reciprocal(rc[:], us[:])
                rcg = small.tile([P, 1], F32, tag="sm", name=f"rcg{cb}")
                nc.vector.tensor_mul(rcg[:], rc[:], gh[:])
                nc.vector.tensor_scalar_mul(
                    attn_n[:, cb, :], attn_n[:, cb, :], rcg[:]
                )
            for dd in range(2):
                for cc in range(2):
                    t_ps = psO.tile([P, 512], F32, tag="o", name=f"t_{dd}{cc}")[
                        :, :P
                    ]
                    nc.tensor.transpose(
                        t_ps, attn_n[:, cc, dd * P : (dd + 1) * P], ident[:]
                    )
                    nc.scalar.copy(attnTg[:, dd, cc * P : (cc + 1) * P], t_ps)

            # ---- channel-branch output rows, precomputed into SBUF ----
            # outc_sb[blk] = gamma/2 * attn_c @ x2_rows(blk) + x2_rows(blk)
            # (residual folded in here so the main-loop epilogue is 2 DVE ops)
            outc_sb = persist.tile([P, NBLK, C], F32, tag="outc_sb")
            for blk in range(NBLK):
                c_ps = psS.tile([P, 1024], F32, tag="s", name=f"c_{blk}")[:, :C]
                for dd in range(2):
                    nc.tensor.matmul(
                        c_ps,
                        x2T[:, dd, blk * P : (blk + 1) * P],
                        attnTg[:, dd, :],
                        start=(dd == 0),
                        stop=(dd == 1),
                    )
                nc.vector.tensor_add(
                    outc_sb[:, blk, :], c_ps, x2[:, blk, :]
                )

            # ---------------- Phase D: point attention ----------------
            # Software-pipelined: scores/exp for round i are emitted BEFORE
            # the PV matmuls of round i-1 so the (in-order) PE stream always
            # has score work to do while ACT computes the exp it needs next.
            RPM = KCH // 2  # rounds per macro block (2 key chunks per round)
            NM = HALF // 512  # macro blocks
            o_ps: dict = {}
            pT_t: dict = {}

            def emit_scores(m, r):
                s_ps = psS.tile([P, 1024], F32, tag="s", name=f"s_{m}_{r}")
                for half in range(2):
                    # group `half` holds chunk kk=2r+half; the two matmuls
                    # occupy PE row groups 0/32 and run concurrently
                    nc.tensor.matmul(
                        s_ps[:, half * 512 : (half + 1) * 512],
                        kT2[half * C8 : (half + 1) * C8, r * P : (r + 1) * P],
                        qT2[half * C8 : (half + 1) * C8, m * 512 : (m + 1) * 512],
                        start=True,
                        stop=True,
                    )
                pT = pTp.tile([P, 1024], BF16, tag="pT", name=f"p_{m}_{r}")
                nc.scalar.activation(pT[:], s_ps[:], ACTF.Exp)
                pT_t[(m, r)] = pT

            def emit_pv(m, r):
                pT = pT_t.pop((m, r))
                for half in range(2):
                    kk = 2 * r + half
                    for j in range(4):
                        nc.tensor.matmul(
                            o_ps[(m, j)],
                            pT[:, half * 512 + j * P : half * 512 + (j + 1) * P],
                            vaug[:, kk, :],
                            start=(kk == 0),
                            stop=(kk == KCH - 1),
                        )

            def emit_epilogue(m):
                for j in range(4):
                    blk = m * 4 + j
                    # single fast PSUM read frees the O bank for the next
                    # macro block's PV accumulation almost immediately
                    osb = sbout.tile([P, C + 2], F32, tag="osb", name=f"osb{blk}")
                    nc.vector.tensor_copy(osb[:], o_ps.pop((m, j)))
                    rq = small.tile([P, 1], F32, tag="sm", name=f"rq{blk}")
                    nc.vector.reciprocal(rq[:], osb[:, C : C + 1])
                    rqg = small.tile([P, 1], F32, tag="sm", name=f"rqg{blk}")
                    nc.vector.tensor_mul(rqg[:], rq[:], gb[:])
                    acc = sbout.tile([P, C], F32, tag="acc", name=f"acc{blk}")
                    nc.vector.tensor_scalar_mul(acc[:], osb[:, :C], rqg[:])
                    nc.vector.tensor_add(acc[:], acc[:], outc_sb[:, blk, :])
                    nc.sync.dma_start(out_d.ap()[blk], acc[:])

            rounds = [(m, r) for m in range(NM) for r in range(RPM)]
            for i, (m, r) in enumerate(rounds):
                if r == 0:
                    for j in range(4):
                        o_ps[(m, j)] = psO.tile(
                            [P, 512], F32, tag="o", name=f"o_{m}_{j}"
                        )[:, : C + 2]
                emit_scores(m, r)
                if i > 0:
                    pm, pr = rounds[i - 1]
                    emit_pv(pm, pr)
                    if pr == RPM - 1:
                        emit_epilogue(pm)
            emit_pv(*rounds[-1])
            emit_epilogue(NM - 1)

    nc.compile()
    return nc


def _prep_core_inputs(points, Wq, bq, Wk, bk, Wv, bv, gamma, core):
    b, h = core // 2, core % 2
    xb = np.asarray(points[b], dtype=np.float32)
    # own rows first, then the other half (key order is softmax-invariant
    # as long as kT and v use the same order, which they do)
    xp = np.concatenate([xb[h * HALF : (h + 1) * HALF], xb[(1 - h) * HALF : (2 - h) * HALF]])
    x2 = (2.0 * xp).reshape(KCH, P, C).transpose(1, 0, 2).astype(np.float16)  # (128, 32, 256)
    x2T = np.ascontiguousarray(
        (2.0 * xp).T.reshape(2, P, N).transpose(1, 0, 2)
    ).astype(np.float16)  # (128, 2, 4096)
    return {"x2T": x2T, "x2": x2}


def _prep_shared_inputs(Wq, bq, Wk, bk, Wv, bv, gamma):
    wqT = np.ascontiguousarray((0.5 * np.asarray(Wq, np.float32).T).reshape(2, P, C8).transpose(1, 0, 2)).astype(np.float16)
    wkT = np.ascontiguousarray((0.5 * np.asarray(Wk, np.float32).T).reshape(2, P, C8).transpose(1, 0, 2)).astype(np.float16)
    wvT_full = 0.5 * np.asarray(Wv, np.float32).T  # (256, 256)
    wvT_aug = np.concatenate(
        [wvT_full, np.zeros((C, 2), np.float32)], axis=1
    )  # (256, 258)
    wvT = np.ascontiguousarray(wvT_aug.reshape(2, P, C + 2).transpose(1, 0, 2)).astype(np.float16)
    bv_aug = np.concatenate([np.asarray(bv, np.float32), [1.0, 0.0]]).reshape(1, C + 2)
    return {
        "wqT": wqT,
        "wkT": wkT,
        "wvT": wvT,
        "bqc2": np.concatenate(
            [np.asarray(bq, np.float32), np.asarray(bq, np.float32)]
        ).reshape(2 * C8, 1),
        "bkc2": np.concatenate(
            [np.asarray(bk, np.float32), np.asarray(bk, np.float32)]
        ).reshape(2 * C8, 1),
        "bvr": bv_aug,
        "gam": np.asarray(gamma, np.float32).reshape(1, 1),
    }


def kernel(points, Wq, bq, Wk, bk, Wv, bv, gamma, **run_kwargs):
    if "nc" not in _CACHE:
        _CACHE["nc"] = _build_nc()
    nc = _CACHE["nc"]

    shared = _prep_shared_inputs(Wq, bq, Wk, bk, Wv, bv, gamma)
    in_maps = []
    for core in range(NCORES):
        m = dict(shared)
        m.update(_prep_core_inputs(points, Wq, bq, Wk, bk, Wv, bv, gamma, core))
        in_maps.append(m)

    res = run_bass_kernel_spmd(
        nc, in_maps, core_ids=list(range(NCORES)), **run_kwargs
    )
    out = np.empty((B, N, C), dtype=np.float32)
    for core in range(NCORES):
        b, h = core // 2, core % 2
        out[b, h * HALF : (h + 1) * HALF] = (
            res.results[core]["out_rows"].reshape(HALF, C)
        )
    if run_kwargs:
        kernel.last_results = res  # expose profile info to test harness
    return out
